# revision 41
# baseline (speedup 1.0000x reference)
"""MoE SwiGLU experts kernel for Trainium2 (8 NeuronCores, expert-parallel).

Each core owns one expert. Host dispatches tokens (gather + dedup with summed
combine weights), repacks every operand into SBUF-exact DRAM layouts so each
DMA is 128 descriptors of >=4KB contiguous bytes (full 360 GB/s, no
read-modify-write penalty), and pads the token batch to capacity C.

Device schedule (single serialized DMA device in the TRN2 cost model):
- All input DMAs are issued on the Pool/SWDGE queue in exactly the order the
  PE consumes them: x-chunk0 (small, 256 tokens), then gw/uw im-column-slices
  interleaved, then the remaining x chunks, then dw hm-slices. The PE starts
  ~5.5us in and never starves.
- Phase 1 runs chunk-outer / im-inner so only the first small x chunk gates
  the start:   interT[:, c] = silu(gw @ xT[:, c]) * (uw @ xT[:, c])
- Phase 2 computes outT = dw @ interT per hm row-block; ACT evacuates each
  PSUM chunk into an SBUF arena aliased over the (dead) x slab, and a per-hm
  DMA stores it out, keeping the kernel tail short.
"""

import numpy as np

P = 128


def _p1_chunks(C):
    # phase-1 token chunks: all >=256 (DMA full-rate needs >=512B runs) and
    # <=512 (one PSUM bank). First chunk small so the PE can start early.
    if C <= 512:
        return [(0, C)]
    if C <= 768:
        return [(0, 256), (256, C - 256)]
    return [(0, 256), (256, C - 512), (C - 256, 256)]


def _p2_chunks(C):
    if C <= 512:
        return [(0, C)]
    return [(0, 512), (512, C - 512)]


def _p2_tail_chunks(C):
    # final row-block: taper so the last chunk (and its evac + store) is small
    if C <= 512:
        return [(0, C)]
    r = C - 512
    return [(0, 512), (512, r)]


WARMUP_K = 25


def _build_bass(C: int, H: int, I: int):
    from contextlib import ExitStack

    import concourse.bass as bass
    import concourse.mybir as mybir
    import concourse.tile as tile

    f32 = mybir.dt.float32
    bf16 = mybir.dt.bfloat16
    KH = H // P  # 16
    KI = I // P  # 11
    FW = KH * P  # 2048 free bytes-per-im-slice elems
    FD = KI * P

    p1c = _p1_chunks(C)
    p2c = _p2_chunks(C)

    nc = bass.Bass(dynamic_dma_scratch_size=8192)
    xR_d = nc.dram_tensor("xR", [P, KH * C], bf16, kind="ExternalInput")
    gwR_d = nc.dram_tensor("gwR", [KI, P, FW], bf16, kind="ExternalInput")
    uwR_d = nc.dram_tensor("uwR", [KI, P, FW], bf16, kind="ExternalInput")
    dwR_d = nc.dram_tensor("dwR", [KH, P, FD], bf16, kind="ExternalInput")
    outR_d = nc.dram_tensor("outR", [KH, P, C], bf16, kind="ExternalOutput")

    gw3 = gwR_d[:]
    uw3 = uwR_d[:]
    dw3 = dwR_d[:]
    xr2 = xR_d[:]
    o3 = outR_d[:]
    o3r = outR_d[:].rearrange("h p c -> p h c")

    with ExitStack() as ctx:
        tc = ctx.enter_context(tile.TileContext(nc))
        wpool = ctx.enter_context(tc.tile_pool(name="w", bufs=1))
        ipool = ctx.enter_context(tc.tile_pool(name="inter", bufs=1))
        ppool = ctx.enter_context(tc.tile_pool(name="psum1", bufs=2, space="PSUM"))
        ppool2 = ctx.enter_context(tc.tile_pool(name="psum2", bufs=2, space="PSUM"))
        spool = ctx.enter_context(tc.tile_pool(name="staging", bufs=4))

        gw_sb = wpool.tile([P, KI, FW], bf16, name="gw_sb")
        uw_sb = wpool.tile([P, KI, FW], bf16, name="uw_sb")
        dw_sb = wpool.tile([P, KH, FD], bf16, name="dw_sb")
        inter_sb = ipool.tile([P, KI, C], bf16, name="inter_sb")
        puc_sb = ipool.tile([P, KI, C], bf16, name="puc_sb")
        # x and the output staging buffer share one slab: x is dead once
        # phase 1 finishes, and every out write is transitively ordered
        # after every x read via the PE program order.
        xo_arena = nc.alloc_sbuf_tensor("xo_arena", [P, KH * C], bf16)
        xo_off = nc.lookup_mloc(xo_arena).addr
        x_flat = nc.alloc_sbuf_tensor_at("x_flat", [P, KH * C], bf16, offset=xo_off)[:]
        out_view = nc.alloc_sbuf_tensor_at(
            "out_view", [P, KH, C], bf16, offset=xo_off
        )[:]
        # garbage-value warmup tile: a tiny matmul right at kernel start takes
        # the PE out of its low p-state before the first real matmul arrives
        warm_sb = wpool.tile([P, 256], bf16, name="warm_sb")

        from concourse.tile import add_dep_helper

        # ---- DMA issue order == PE consumption order ----
        # first two pieces via HWDGE (lower launch latency than SWDGE);
        # everything else on the SWDGE queue in consumption order
        x_dmas = []
        gw_dmas = []
        uw_dmas = []
        dw_dmas = []
        (o0, l0) = p1c[0]
        x_dmas.append(nc.sync.dma_start(
            x_flat[:, KH * o0 : KH * (o0 + l0)], xr2[:, KH * o0 : KH * (o0 + l0)]
        ))
        for im in range(KI):
            gw_dmas.append(nc.gpsimd.dma_start(gw_sb[:, im], gw3[im]))
            uw_dmas.append(nc.gpsimd.dma_start(uw_sb[:, im], uw3[im]))
        for (off, l) in p1c[1:]:
            x_dmas.append(nc.gpsimd.dma_start(
                x_flat[:, KH * off : KH * (off + l)], xr2[:, KH * off : KH * (off + l)]
            ))
        for hm in range(KH):
            dw_dmas.append(nc.gpsimd.dma_start(dw_sb[:, hm], dw3[hm]))
        load_dmas = gw_dmas + uw_dmas + dw_dmas

        # single-edge PE fence: absorbs one producer's semaphore into the PE
        # stream so the matmuls that follow never carry a DMA/DVE wait of
        # their own (walrus allows roughly one sync wait per instruction)
        def pe_fence(target):
            f = nc.tensor.ldweights(warm_sb[:, 0:1])
            add_dep_helper(f.ins, target.ins, sync=True, reason="pe fence")

        # ---- warmup chain: keeps the PE busy-streak alive through the
        # initial DMA wait so the real matmuls are all costed at the full
        # ramped p-state (the cost model derates the first ~3us of a streak)
        if WARMUP_K:
            nc.vector.memset(warm_sb[:], 1.0)
            wp = ppool.tile([P, 512], f32, tag="g", name="pg")
            for _ in range(WARMUP_K):
                nc.tensor.matmul(wp[:, 0:256], warm_sb[:, 0:128],
                                 warm_sb[:, 0:256], start=True, stop=True)

        # ---- phase 1: interT = silu(gwT x) * (uwT x), chunk-outer ----
        last_mul = [None]
        chunk_last_mul = []
        for ci, (off, l) in enumerate(p1c):
            base = KH * off
            pe_fence(x_dmas[ci])
            for im in range(KI):
                pg = ppool.tile([P, 512], f32, tag="g", name="pg")
                pu = ppool.tile([P, 512], f32, tag="u", name="pu")
                if ci == 0:
                    pe_fence(gw_dmas[im])
                for kh in range(KH):
                    nc.tensor.matmul(
                        pg[:, :l],
                        gw_sb[:, im, kh * P : (kh + 1) * P],
                        x_flat[:, base + kh * l : base + (kh + 1) * l],
                        start=(kh == 0),
                        stop=(kh == KH - 1),
                    )
                if ci == 0:
                    pe_fence(uw_dmas[im])
                for kh in range(KH):
                    nc.tensor.matmul(
                        pu[:, :l],
                        uw_sb[:, im, kh * P : (kh + 1) * P],
                        x_flat[:, base + kh * l : base + (kh + 1) * l],
                        start=(kh == 0),
                        stop=(kh == KH - 1),
                    )
                # ACT evacuates both PSUM tiles into write-once SBUF regions
                # ({PE} is each copy's only wait); the DVE multiply then reads
                # two ACT products — a single semaphore, which is all a DVE
                # TensorTensor instruction can wait on
                nc.scalar.activation(
                    inter_sb[:, im, off : off + l],
                    pg[:, :l],
                    mybir.ActivationFunctionType.Silu,
                )
                nc.scalar.copy(puc_sb[:, im, off : off + l], pu[:, :l])
                last_mul[0] = nc.vector.tensor_mul(
                    inter_sb[:, im, off : off + l],
                    inter_sb[:, im, off : off + l],
                    puc_sb[:, im, off : off + l],
                )
            chunk_last_mul.append(last_mul[0])

        # ---- alias fences: out_view overlays the x slab, so its writers
        # (ACT copies) and readers (SP stores) inherit WAW/RAW deps on the
        # x DMAs. Absorb those once into each engine's observed-sem set via
        # single-edge fence ops so the real copies/stores keep one wait each.
        fence_t = spool.tile([P, 16], bf16, tag="fence", name="fence_t")
        for i, d in enumerate(x_dmas):
            fa = nc.scalar.copy(
                fence_t[:1, 4 * i : 4 * i + 4], fence_t[:1, 4 * i : 4 * i + 4]
            )
            add_dep_helper(fa.ins, d.ins, sync=True, reason="x alias fence act")
        # sacrificial DRAM->DRAM DMAs on HWDGE: each takes one SWDGE x-DMA
        # wait, so the real output stores (FIFO-ordered behind them on the
        # HWDGE queue) don't carry the arena-alias RAW waits themselves — a
        # DMA has one wait slot, which the stores spend on their ACT-copy
        # producer. x-c0 itself is HWDGE, covered by the same FIFO order.
        scr_d = nc.dram_tensor("scr", [8, 16], bf16)
        dummy_stores = []
        for i, d in enumerate(x_dmas):
            ds = nc.sync.dma_start(scr_d[:][2 + i : 3 + i, :], scr_d[:][0:1, :])
            add_dep_helper(ds.ins, d.ins, sync=True, reason="x alias dummy hw")
            dummy_stores.append(ds)


        # phase-2 matmuls read inter regions produced by the DVE multiplies;
        # absorb those sems at the phase boundary (per phase-2 chunk, so the
        # first chunk's matmuls don't wait for the very last multiply)
        def p1_cover(off2, l2):
            k = 0
            for ci, (off, l) in enumerate(p1c):
                if off < off2 + l2 and off + l > off2:
                    k = ci
            return chunk_last_mul[k]

        # ---- phase 2: outT = dwT @ interT, hm-outer ----
        store_dmas = []
        tail_insts = []
        for hm in range(KH):
            last = hm == KH - 1
            # the final row-block tapers into smaller chunks so the closing
            # evacuation + store chain is as short as possible
            chunks2 = _p2_tail_chunks(C) if last else p2c
            pos = [
                ppool2.tile([P, 512], f32, tag=f"o{j % 2}", name=f"po{j % 2}")
                for j in range(len(chunks2))
            ]
            pe_fence(dw_dmas[hm])
            for j, (off, l) in enumerate(chunks2):
                if hm == 0:
                    pe_fence(p1_cover(off, l))
                for ki in range(KI):
                    nc.tensor.matmul(
                        pos[j][:, :l],
                        dw_sb[:, hm, ki * P : (ki + 1) * P],
                        inter_sb[:, ki, off : off + l],
                        start=(ki == 0),
                        stop=(ki == KI - 1),
                    )
                tail_insts.append(
                    nc.scalar.copy(out_view[:, hm, off : off + l], pos[j][:, :l])
                )
                if last:
                    # final row-block: store each tapered chunk as soon as
                    # its evacuation lands, keeping the kernel tail short
                    store_dmas.append(nc.sync.dma_start(
                        o3[hm][:, off : off + l], out_view[:, hm, off : off + l]
                    ))
            # bulk rows ship in two big HWDGE stores (after hm 11 and 14):
            # together with the x-c0 load, the alias dummies, and the three
            # tail stores that is exactly 8 HWDGE DMAs — more would draw
            # semaphore-rotation waits, and a DMA's one wait slot is spent
            # on its ACT-copy producer
            if hm == 11:
                store_dmas.append(nc.sync.dma_start(
                    o3r[:, 0:12], out_view[:, 0:12]
                ))
            elif hm == KH - 2:
                store_dmas.append(nc.sync.dma_start(
                    o3r[:, 12 : KH - 1], out_view[:, 12 : KH - 1]
                ))

        # ---- pre-drain: absorb every dangling producer's final tick into
        # the SP sequencer one sync edge at a time, so the kernel-tail drain
        # (which would otherwise need more wait slots than the instruction
        # has) carries no waits of its own
        for insts in (load_dmas, x_dmas, dummy_stores, store_dmas,
                      [last_mul[0]], tail_insts[-2:]):
            for bi in insts:
                if bi is None:
                    continue
                nop = nc.sync.nop()
                add_dep_helper(nop.ins, bi.ins, sync=True, reason="pre-drain")

    return nc


def kernel(hidden_states, top_k_index, top_k_weights, gate_w, up_w, down_w):
    import ml_dtypes
    from concourse.bass_utils import run_bass_kernel_spmd

    bf = ml_dtypes.bfloat16
    hs = np.ascontiguousarray(np.asarray(hidden_states, dtype=np.float32))
    tki = np.asarray(top_k_index)
    tkw = np.asarray(top_k_weights, dtype=np.float32)
    gw = np.asarray(gate_w, dtype=np.float32)
    uw = np.asarray(up_w, dtype=np.float32)
    dw = np.asarray(down_w, dtype=np.float32)

    T, H = hs.shape
    E, I, _ = gw.shape
    KH, KI = H // P, I // P

    tok_lists, w_lists = [], []
    for e in range(E):
        mask = tki == e
        toks = np.nonzero(mask.any(axis=1))[0]
        w = (tkw * mask).sum(axis=1)[toks].astype(np.float32)
        tok_lists.append(toks)
        w_lists.append(w)

    # capacity = the actual max expert load (rounded up); experts that
    # somehow exceed it spill into additional SPMD rounds
    C = max(256, -(-max(len(t) for t in tok_lists) // 8) * 8)
    C = min(C, 1024)
    n_rounds = max(1, -(-max(len(t) for t in tok_lists) // C))
    p1c = _p1_chunks(C)

    def pack_w(w_eih):  # [I, H] -> [KI, P, KH*P], SBUF-exact im-slices
        return np.ascontiguousarray(
            w_eih.astype(bf).reshape(KI, P, KH, P).transpose(0, 3, 2, 1)
            .reshape(KI, P, KH * P)
        )

    def pack_d(w_ehi):  # [H, I] -> [KH, P, KI*P], SBUF-exact hm-slices
        return np.ascontiguousarray(
            w_ehi.astype(bf).reshape(KH, P, KI, P).transpose(0, 3, 2, 1)
            .reshape(KH, P, KI * P)
        )

    gwR = [pack_w(gw[e]) for e in range(E)]
    uwR = [pack_w(uw[e]) for e in range(E)]
    dwR = [pack_d(dw[e]) for e in range(E)]

    nc = _build_bass(C, H, I)
    out = np.zeros((T, H), np.float32)
    global _last_results, _last_nc, _last_C
    _last_nc, _last_C = nc, C
    for r in range(n_rounds):
        in_maps = []
        for e in range(E):
            toks = tok_lists[e][r * C : (r + 1) * C]
            x_pad = np.zeros((C, H), np.float32)
            x_pad[: len(toks)] = hs[toks]
            xb = x_pad.astype(bf)
            # chunk-major blocks: [P, sum_over_chunks(KH*l)], each block
            # [p, kh, t] so a chunk DMA is one 128-descriptor transfer
            xR = np.concatenate(
                [
                    xb[off : off + l].reshape(l, KH, P).transpose(2, 1, 0)
                    .reshape(P, KH * l)
                    for (off, l) in p1c
                ],
                axis=1,
            )
            in_maps.append(
                {
                    "xR": np.ascontiguousarray(xR),
                    "gwR": gwR[e],
                    "uwR": uwR[e],
                    "dwR": dwR[e],
                }
            )
        res = run_bass_kernel_spmd(nc, in_maps, core_ids=list(range(E)))
        _last_results = res
        for e in range(E):
            toks = tok_lists[e][r * C : (r + 1) * C]
            n = len(toks)
            if n == 0:
                continue
            outT_e = (
                np.asarray(res.results[e]["outR"]).astype(np.float32).reshape(H, C)
            )
            out[toks] += w_lists[e][r * C : r * C + n, None] * outT_e[:, :n].T
    return out
